# revision 17
# baseline (speedup 1.0000x reference)
"""AudioX MMDiT self-attention Trainium2 kernel.

Sharding: 8 cores = batch(2) x head-group(4). Each core handles one batch row
and 4 of the 16 heads; everything (qkv proj, RMSNorm, RoPE, SDPA) is local.

Layout strategy per core:
  - token-major QKV projection (tokens on partitions) so RMSNorm reduction and
    RoPE pair-mixing live on the free axis,
  - PE transposes q,k to feature-major for the scores matmul,
  - scores computed transposed (S_T[nk, nq]) so softmax denominator and AV both
    contract over partitions; denominator comes free via a ones-augmented V
    column,
  - exp on ScalarE without max-subtraction (scores bounded; verified on data),
  - final AV output transposed back to token-major and normalized.

The feature order of q,k is permuted host-side (even pairs first) so RoPE is a
rotate-half; the same permutation on q and k leaves scores unchanged.
"""

import numpy as np

B, N, DIM, H = 2, 2048, 1024, 16
D = DIM // H          # 64
HL = 4                # heads per core
EPS = 1e-6
NT = N // 128         # 16 token tiles
KT = DIM // 128       # 8 contraction tiles
FQ = HL * D           # 256 features per q/k/v group

_COMPILED = None


def _build_bass():
    import concourse.bacc as bacc
    import concourse.mybir as mybir
    import concourse.tile as tile
    import concourse.bass as bass

    fp32 = mybir.dt.float32
    f32r = mybir.dt.float32r
    AF = mybir.ActivationFunctionType
    ALU = mybir.AluOpType
    AX = mybir.AxisListType

    nc = bacc.Bacc("TRN2", target_bir_lowering=False, debug=False, num_devices=8)

    xT_d = nc.dram_tensor("xT", [DIM, N], f32r, kind="ExternalInput").ap()
    w_d = nc.dram_tensor("w768", [DIM, 3 * FQ], f32r, kind="ExternalInput").ap()
    bias_d = nc.dram_tensor("bias768", [1, 3 * FQ], f32r, kind="ExternalInput").ap()
    ad_d = nc.dram_tensor("adtab", [N, FQ], fp32, kind="ExternalInput").ap()
    bc_d = nc.dram_tensor("bctab", [N, FQ], fp32, kind="ExternalInput").ap()
    idn_d = nc.dram_tensor("idn", [128, 128], f32r, kind="ExternalInput").ap()
    on1_d = nc.dram_tensor("ones1", [1, 128], f32r, kind="ExternalInput").ap()
    onc_d = nc.dram_tensor("onescol", [128, NT * HL], f32r, kind="ExternalInput").ap()
    out_d = nc.dram_tensor("out_local", [N, FQ], fp32, kind="ExternalOutput").ap()

    with tile.TileContext(nc) as tc:
        with tc.tile_pool(name="const", bufs=1) as cpool:
            # resident inputs
            xT = cpool.tile([128, KT, N], f32r, tag="xT")
            nc.sync.dma_start(xT[:], xT_d.rearrange("(kt p) n -> p kt n", p=128))
            w = cpool.tile([128, KT, 3 * FQ], f32r, tag="w")
            nc.sync.dma_start(w[:], w_d.rearrange("(kt p) f -> p kt f", p=128))
            brow = cpool.tile([1, 3 * FQ], f32r, tag="brow")
            nc.sync.dma_start(brow[:], bias_d[:])
            idn = cpool.tile([128, 128], f32r, tag="idn")
            nc.sync.dma_start(idn[:], idn_d[:])
            ones1 = cpool.tile([1, 128], f32r, tag="ones1")
            nc.sync.dma_start(ones1[:], on1_d[:])
            epsb = cpool.tile([128, 1], fp32, tag="epsb")
            nc.vector.memset(epsb[:], EPS)

            # persistent intermediates
            vaug = cpool.tile([128, NT, HL, D + 1], f32r, tag="vaug")
            nc.sync.dma_start(
                vaug[:, :, :, D:D + 1],
                onc_d[:].rearrange("p (nt h one) -> p nt h one", h=HL, one=1))
            qf = cpool.tile([128, 2, N], f32r, tag="qf")   # [2 pair-tiles]
            kf = cpool.tile([128, 2, N], f32r, tag="kf")

            # ---------------- phase 1: projection + norm + rope -------------
            with (
                tc.tile_pool(name="proj_ps", bufs=2, space="PSUM") as pps,
                tc.tile_pool(name="proj_sb", bufs=3) as psb,
                tc.tile_pool(name="tr_ps", bufs=2, space="PSUM") as tps,
            ):
                for nt in range(NT):
                    adt = psb.tile([128, FQ], fp32, tag="adt")
                    nc.sync.dma_start(adt[:], ad_d[nt * 128:(nt + 1) * 128, :])
                    bct = psb.tile([128, FQ], fp32, tag="bct")
                    nc.sync.dma_start(bct[:], bc_d[nt * 128:(nt + 1) * 128, :])
                    qkv_ps = []
                    for g in range(3):  # q, k, v groups of 256 cols
                        ps = pps.tile([128, FQ], fp32, tag=f"ps{g}")
                        for kt in range(KT):
                            nc.tensor.matmul(
                                ps[:],
                                xT[:, kt, nt * 128:(nt + 1) * 128],
                                w[:, kt, g * FQ:(g + 1) * FQ],
                                start=(kt == 0), stop=False,
                            )
                        # bias via ones-row rank-1 update
                        nc.tensor.matmul(
                            ps[:], ones1[:],
                            brow[:, g * FQ:(g + 1) * FQ],
                            start=False, stop=True,
                        )
                        qkv_ps.append(ps)
                    qp, kp, vp = qkv_ps

                    # v: one strided copy into augmented tile (col D is ones)
                    nc.vector.tensor_copy(
                        vaug[:, nt, :, 0:D],
                        vp[:].rearrange("p (h d) -> p h d", h=HL),
                    )

                    # drain q,k to SBUF (PSUM allows one DVE read port)
                    qs = psb.tile([128, 2 * FQ], fp32, tag="qs")
                    nc.scalar.copy(qs[:, 0:FQ], qp[:])
                    nc.scalar.copy(qs[:, FQ:], kp[:])
                    # sum of squares per head (q|k) -> [128, 8]
                    sq = psb.tile([128, 2 * FQ], fp32, tag="sq")
                    nc.gpsimd.tensor_tensor(sq[:], qs[:], qs[:], ALU.mult)
                    ms = psb.tile([128, 2 * HL], fp32, tag="ms")
                    nc.vector.tensor_reduce(
                        ms[:], sq[:].rearrange("p (g d) -> p g d", d=D),
                        axis=AX.X, op=ALU.add,
                    )
                    rms = psb.tile([128, 2 * HL], fp32, tag="rms")
                    nc.scalar.activation(rms[:], ms[:], AF.Sqrt,
                                         scale=1.0 / D, bias=epsb[:])
                    rinv = psb.tile([128, 2 * HL], fp32, tag="rinv")
                    nc.vector.reciprocal(rinv[:], rms[:])

                    # rope: m1 = qk * [A|D]-table ; m2 = swap(qk) * [B|C]-table
                    qr = psb.tile([128, 2 * FQ], fp32, tag="qr")
                    m2 = psb.tile([128, 2 * FQ], fp32, tag="m2")
                    for gi in range(2):
                        o = gi * FQ
                        gp = qs[:, o:o + FQ]
                        nc.vector.tensor_tensor(
                            qr[:, o:o + FQ], gp, adt[:], ALU.mult)
                        g4 = gp.rearrange("p (h t s) -> p h t s", h=HL, t=2, s=D // 2)
                        m4 = m2[:, o:o + FQ].rearrange(
                            "p (h t s) -> p h t s", h=HL, t=2, s=D // 2)
                        b4 = bct[:].rearrange(
                            "p (h t s) -> p h t s", h=HL, t=2, s=D // 2)
                        # top half gets (bottom input)*B, bottom gets (top)*C
                        nc.gpsimd.tensor_tensor(
                            m4[:, :, 0, :], g4[:, :, 1, :], b4[:, :, 0, :], ALU.mult)
                        nc.gpsimd.tensor_tensor(
                            m4[:, :, 1, :], g4[:, :, 0, :], b4[:, :, 1, :], ALU.mult)
                    nc.gpsimd.tensor_tensor(qr[:], qr[:], m2[:], ALU.add)
                    # apply 1/rms per head, rounding to f32r for the transpose
                    qr_r = psb.tile([128, 2 * FQ], f32r, tag="qr_r")
                    for j in range(2 * HL):
                        nc.vector.tensor_scalar_mul(
                            qr_r[:, j * D:(j + 1) * D], qr[:, j * D:(j + 1) * D],
                            rinv[:, j:j + 1],
                        )

                    # transpose q,k to feature-major
                    for gi, dst in ((0, qf), (1, kf)):
                        tp = tps.tile([128, 2, 128], fp32, tag="tp")
                        for pr in range(2):  # head pairs
                            nc.tensor.transpose(
                                tp[:, pr, :].bitcast(f32r),
                                qr_r[:, gi * FQ + pr * 128: gi * FQ + (pr + 1) * 128],
                                idn[:],
                            )
                            nc.vector.tensor_copy(
                                dst[:, pr, nt * 128:(nt + 1) * 128], tp[:, pr, :])

            # ---------------- phase 2: attention ----------------------------
            import os as _os
            if _os.environ.get("K_PHASE1_ONLY"):
                # debug: stop after projection/norm/rope
                nc_dummy = None
            _skip2 = bool(_os.environ.get("K_PHASE1_ONLY"))
            with (
                tc.tile_pool(name="s_ps", bufs=2, space="PSUM") as sps,
                tc.tile_pool(name="o_ps", bufs=1, space="PSUM") as ops,
                tc.tile_pool(name="t_ps", bufs=2, space="PSUM") as fps,
                tc.tile_pool(name="att_sb", bufs=3) as asb,
            ):
                for pr in range((0 if _skip2 else 2)):        # head pair (2 heads each)
                    for nq in range(4):    # 512-wide query blocks
                        oA = ops.tile([D + 1, 512], fp32, tag="oA")
                        oB = ops.tile([D + 1, 512], fp32, tag="oB")
                        for nkt in range(NT):
                            s = sps.tile([128, 2, 512], fp32, tag="s")
                            nc.tensor.matmul(
                                s[:, 0, :],
                                kf[0:64, pr, nkt * 128:(nkt + 1) * 128],
                                qf[0:64, pr, nq * 512:(nq + 1) * 512],
                                start=True, stop=True, tile_position=(0, 0),
                            )
                            nc.tensor.matmul(
                                s[:, 1, :],
                                kf[64:128, pr, nkt * 128:(nkt + 1) * 128],
                                qf[64:128, pr, nq * 512:(nq + 1) * 512],
                                start=True, stop=True, tile_position=(64, 0),
                            )
                            e = asb.tile([128, 2, 512], f32r, tag="e")
                            nc.scalar.activation(e[:], s[:], AF.Exp, scale=0.125)
                            for hh, o in ((0, oA), (1, oB)):
                                nc.tensor.matmul(
                                    o[:],
                                    vaug[:, nkt, pr * 2 + hh, :],
                                    e[:, hh, :],
                                    start=(nkt == 0), stop=(nkt == NT - 1),
                                )
                        # transpose back to token-major + normalize + store
                        for hh, o in ((0, oA), (1, oB)):
                            h = pr * 2 + hh
                            osb = asb.tile([128, 512], f32r, tag="osb")
                            nc.vector.tensor_copy(osb[0:D + 1, :], o[:])
                            for sub in range(4):
                                ot = fps.tile([128, 128], fp32, tag="ot")
                                nc.tensor.transpose(
                                    ot[:].bitcast(f32r),
                                    osb[:, sub * 128:(sub + 1) * 128],
                                    idn[:],
                                )
                                rc = asb.tile([128, 1], fp32, tag="rc")
                                nc.vector.reciprocal(rc[:], ot[:, D:D + 1])
                                ob = asb.tile([128, D], fp32, tag="ob")
                                nc.vector.tensor_scalar_mul(ob[:], ot[:, 0:D], rc[:])
                                nt_o = nq * 4 + sub
                                nc.sync.dma_start(
                                    out_d[nt_o * 128:(nt_o + 1) * 128,
                                          h * D:(h + 1) * D],
                                    ob[:],
                                )
    nc.compile()
    return nc


def _prep_inputs(x, W_qkv, b_qkv, rot):
    """Build the 8 per-core input maps."""
    x = np.ascontiguousarray(x, np.float32)
    W = np.asarray(W_qkv, np.float32)
    bq = np.asarray(b_qkv, np.float32)
    rot = np.asarray(rot, np.float32).reshape(N, D // 2, 2, 2)

    # feature permutation for q/k: even pair-components first, then odd
    dperm = np.concatenate([np.arange(0, D, 2), np.arange(1, D, 2)])

    # rope tables, token-major, replicated across the 4 local heads
    A = rot[:, :, 0, 0]; Bc = rot[:, :, 0, 1]
    C = rot[:, :, 1, 0]; Dd = rot[:, :, 1, 1]
    ad1 = np.concatenate([A, Dd], axis=1)    # [N, 64]
    bc1 = np.concatenate([Bc, C], axis=1)
    adtab = np.tile(ad1, (1, HL)).astype(np.float32)   # [N, 256]
    bctab = np.tile(bc1, (1, HL)).astype(np.float32)
    idn = np.eye(128, dtype=np.float32)

    in_maps = []
    for core in range(8):
        b = core // 4
        g = core % 4
        heads = np.arange(g * HL, (g + 1) * HL)
        cols_q = np.concatenate(
            [h * (D * 3) + dperm * 3 + 0 for h in heads])
        cols_k = np.concatenate(
            [h * (D * 3) + dperm * 3 + 1 for h in heads])
        cols_v = np.concatenate(
            [h * (D * 3) + np.arange(D) * 3 + 2 for h in heads])
        cols = np.concatenate([cols_q, cols_k, cols_v])
        in_maps.append({
            "xT": np.ascontiguousarray(x[b].T),
            "w768": np.ascontiguousarray(W[:, cols]),
            "bias768": np.ascontiguousarray(bq[cols][None, :]),
            "adtab": adtab,
            "bctab": bctab,
            "idn": idn,
            "ones1": np.ones((1, 128), np.float32),
            "onescol": np.ones((128, NT * HL), np.float32),
        })
    return in_maps


def _run(in_maps, trace=False):
    global _COMPILED
    from concourse import bass_utils
    if _COMPILED is None:
        _COMPILED = _build_bass()
    return bass_utils.run_bass_kernel_spmd(
        _COMPILED, in_maps, list(range(8)), trace=trace)


def kernel(x, W_qkv, b_qkv, rot):
    in_maps = _prep_inputs(x, W_qkv, b_qkv, rot)
    res = _run(in_maps)
    out = np.empty((B, N, DIM), np.float32)
    for core in range(8):
        b = core // 4
        g = core % 4
        out[b, :, g * HL * D:(g + 1) * HL * D] = res.results[core]["out_local"]
    return out


# revision 18
# speedup vs baseline: 1.0046x; 1.0046x over previous
"""AudioX MMDiT self-attention Trainium2 kernel.

Sharding: 8 cores = batch(2) x head-group(4). Each core handles one batch row
and 4 of the 16 heads; everything (qkv proj, RMSNorm, RoPE, SDPA) is local.

Layout strategy per core:
  - token-major QKV projection (tokens on partitions) so RMSNorm reduction and
    RoPE pair-mixing live on the free axis,
  - PE transposes q,k to feature-major for the scores matmul,
  - scores computed transposed (S_T[nk, nq]) so softmax denominator and AV both
    contract over partitions; denominator comes free via a ones-augmented V
    column,
  - exp on ScalarE without max-subtraction (scores bounded; verified on data),
  - final AV output transposed back to token-major and normalized.

The feature order of q,k is permuted host-side (even pairs first) so RoPE is a
rotate-half; the same permutation on q and k leaves scores unchanged.
"""

import numpy as np

B, N, DIM, H = 2, 2048, 1024, 16
D = DIM // H          # 64
HL = 4                # heads per core
EPS = 1e-6
NT = N // 128         # 16 token tiles
KT = DIM // 128       # 8 contraction tiles
FQ = HL * D           # 256 features per q/k/v group

_COMPILED = None


def _build_bass():
    import concourse.bacc as bacc
    import concourse.mybir as mybir
    import concourse.tile as tile
    import concourse.bass as bass

    fp32 = mybir.dt.float32
    f32r = mybir.dt.float32r
    AF = mybir.ActivationFunctionType
    ALU = mybir.AluOpType
    AX = mybir.AxisListType

    nc = bacc.Bacc("TRN2", target_bir_lowering=False, debug=False, num_devices=8)

    xT_d = nc.dram_tensor("xT", [DIM, N], f32r, kind="ExternalInput").ap()
    w_d = nc.dram_tensor("w768", [DIM, 3 * FQ], f32r, kind="ExternalInput").ap()
    bias_d = nc.dram_tensor("bias768", [1, 3 * FQ], f32r, kind="ExternalInput").ap()
    ad_d = nc.dram_tensor("adtab", [N, FQ], fp32, kind="ExternalInput").ap()
    bc_d = nc.dram_tensor("bctab", [N, FQ], fp32, kind="ExternalInput").ap()
    idn_d = nc.dram_tensor("idn", [128, 128], f32r, kind="ExternalInput").ap()
    on1_d = nc.dram_tensor("ones1", [1, 128], f32r, kind="ExternalInput").ap()
    onc_d = nc.dram_tensor("onescol", [128, NT * HL], f32r, kind="ExternalInput").ap()
    out_d = nc.dram_tensor("out_local", [N, FQ], fp32, kind="ExternalOutput").ap()

    with tile.TileContext(nc) as tc:
        with tc.tile_pool(name="const", bufs=1) as cpool:
            # resident inputs
            xT = cpool.tile([128, KT, N], f32r, tag="xT")
            nc.sync.dma_start(xT[:], xT_d.rearrange("(kt p) n -> p kt n", p=128))
            w = cpool.tile([128, KT, 3 * FQ], f32r, tag="w")
            nc.sync.dma_start(w[:], w_d.rearrange("(kt p) f -> p kt f", p=128))
            brow = cpool.tile([1, 3 * FQ], f32r, tag="brow")
            nc.sync.dma_start(brow[:], bias_d[:])
            idn = cpool.tile([128, 128], f32r, tag="idn")
            nc.sync.dma_start(idn[:], idn_d[:])
            ones1 = cpool.tile([1, 128], f32r, tag="ones1")
            nc.sync.dma_start(ones1[:], on1_d[:])
            epsb = cpool.tile([128, 1], fp32, tag="epsb")
            nc.vector.memset(epsb[:], EPS)

            # persistent intermediates
            vaug = cpool.tile([128, NT, HL, D + 1], f32r, tag="vaug")
            nc.sync.dma_start(
                vaug[:, :, :, D:D + 1],
                onc_d[:].rearrange("p (nt h one) -> p nt h one", h=HL, one=1))
            qf = cpool.tile([128, 2, N], f32r, tag="qf")   # [2 pair-tiles]
            kf = cpool.tile([128, 2, N], f32r, tag="kf")

            # ---------------- phase 1: projection + norm + rope -------------
            with (
                tc.tile_pool(name="proj_ps", bufs=2, space="PSUM") as pps,
                tc.tile_pool(name="proj_sb", bufs=3) as psb,
                tc.tile_pool(name="tr_ps", bufs=2, space="PSUM") as tps,
            ):
                for nt in range(NT):
                    adt = psb.tile([128, FQ], fp32, tag="adt")
                    nc.sync.dma_start(adt[:], ad_d[nt * 128:(nt + 1) * 128, :])
                    bct = psb.tile([128, FQ], fp32, tag="bct")
                    nc.sync.dma_start(bct[:], bc_d[nt * 128:(nt + 1) * 128, :])
                    qkv_ps = []
                    for g in range(3):  # q, k, v groups of 256 cols
                        ps = pps.tile([128, FQ], fp32, tag=f"ps{g}")
                        for kt in range(KT):
                            nc.tensor.matmul(
                                ps[:],
                                xT[:, kt, nt * 128:(nt + 1) * 128],
                                w[:, kt, g * FQ:(g + 1) * FQ],
                                start=(kt == 0), stop=False,
                            )
                        # bias via ones-row rank-1 update
                        nc.tensor.matmul(
                            ps[:], ones1[:],
                            brow[:, g * FQ:(g + 1) * FQ],
                            start=False, stop=True,
                        )
                        qkv_ps.append(ps)
                    qp, kp, vp = qkv_ps

                    # v: one strided copy into augmented tile (col D is ones)
                    nc.vector.tensor_copy(
                        vaug[:, nt, :, 0:D],
                        vp[:].rearrange("p (h d) -> p h d", h=HL),
                    )

                    # drain q,k to SBUF (PSUM allows one DVE read port)
                    qs = psb.tile([128, 2 * FQ], fp32, tag="qs")
                    nc.scalar.copy(qs[:, 0:FQ], qp[:])
                    nc.scalar.copy(qs[:, FQ:], kp[:])
                    # sum of squares per head (q|k) -> [128, 8]
                    sq = psb.tile([128, 2 * FQ], fp32, tag="sq")
                    nc.gpsimd.tensor_tensor(sq[:], qs[:], qs[:], ALU.mult)
                    ms = psb.tile([128, 2 * HL], fp32, tag="ms")
                    nc.vector.tensor_reduce(
                        ms[:], sq[:].rearrange("p (g d) -> p g d", d=D),
                        axis=AX.X, op=ALU.add,
                    )
                    rms = psb.tile([128, 2 * HL], fp32, tag="rms")
                    nc.scalar.activation(rms[:], ms[:], AF.Sqrt,
                                         scale=1.0 / D, bias=epsb[:])
                    rinv = psb.tile([128, 2 * HL], fp32, tag="rinv")
                    nc.vector.reciprocal(rinv[:], rms[:])

                    # rope: m1 = qk * [A|D]-table ; m2 = swap(qk) * [B|C]-table
                    qr = psb.tile([128, 2 * FQ], fp32, tag="qr")
                    m2 = psb.tile([128, 2 * FQ], fp32, tag="m2")
                    for gi in range(2):
                        o = gi * FQ
                        gp = qs[:, o:o + FQ]
                        nc.vector.tensor_tensor(
                            qr[:, o:o + FQ], gp, adt[:], ALU.mult)
                        g4 = gp.rearrange("p (h t s) -> p h t s", h=HL, t=2, s=D // 2)
                        m4 = m2[:, o:o + FQ].rearrange(
                            "p (h t s) -> p h t s", h=HL, t=2, s=D // 2)
                        b4 = bct[:].rearrange(
                            "p (h t s) -> p h t s", h=HL, t=2, s=D // 2)
                        # top half gets (bottom input)*B, bottom gets (top)*C
                        nc.gpsimd.tensor_tensor(
                            m4[:, :, 0, :], g4[:, :, 1, :], b4[:, :, 0, :], ALU.mult)
                        nc.gpsimd.tensor_tensor(
                            m4[:, :, 1, :], g4[:, :, 0, :], b4[:, :, 1, :], ALU.mult)
                    nc.gpsimd.tensor_tensor(qr[:], qr[:], m2[:], ALU.add)
                    # apply 1/rms per head, rounding to f32r for the transpose
                    qr_r = psb.tile([128, 2 * FQ], f32r, tag="qr_r")
                    for j in range(2 * HL):
                        nc.vector.tensor_scalar_mul(
                            qr_r[:, j * D:(j + 1) * D], qr[:, j * D:(j + 1) * D],
                            rinv[:, j:j + 1],
                        )

                    # transpose q,k to feature-major
                    for gi, dst in ((0, qf), (1, kf)):
                        tp = tps.tile([128, 2, 128], fp32, tag="tp")
                        for pr in range(2):  # head pairs
                            nc.tensor.transpose(
                                tp[:, pr, :].bitcast(f32r),
                                qr_r[:, gi * FQ + pr * 128: gi * FQ + (pr + 1) * 128],
                                idn[:],
                            )
                            nc.vector.tensor_copy(
                                dst[:, pr, nt * 128:(nt + 1) * 128], tp[:, pr, :])

            # ---------------- phase 2: attention ----------------------------
            import os as _os
            if _os.environ.get("K_PHASE1_ONLY"):
                # debug: stop after projection/norm/rope
                nc_dummy = None
            _skip2 = bool(_os.environ.get("K_PHASE1_ONLY"))
            with (
                tc.tile_pool(name="s_ps", bufs=2, space="PSUM") as sps,
                tc.tile_pool(name="o_ps", bufs=1, space="PSUM") as ops,
                tc.tile_pool(name="t_ps", bufs=2, space="PSUM") as fps,
                tc.tile_pool(name="att_sb", bufs=4) as asb,
            ):
                for pr in range((0 if _skip2 else 2)):        # head pair (2 heads each)
                    for nq in range(4):    # 512-wide query blocks
                        oA = ops.tile([D + 1, 512], fp32, tag="oA")
                        oB = ops.tile([D + 1, 512], fp32, tag="oB")
                        for nkt in range(NT):
                            s = sps.tile([128, 2, 512], fp32, tag="s")
                            nc.tensor.matmul(
                                s[:, 0, :],
                                kf[0:64, pr, nkt * 128:(nkt + 1) * 128],
                                qf[0:64, pr, nq * 512:(nq + 1) * 512],
                                start=True, stop=True, tile_position=(0, 0),
                            )
                            nc.tensor.matmul(
                                s[:, 1, :],
                                kf[64:128, pr, nkt * 128:(nkt + 1) * 128],
                                qf[64:128, pr, nq * 512:(nq + 1) * 512],
                                start=True, stop=True, tile_position=(64, 0),
                            )
                            e = asb.tile([128, 2, 512], f32r, tag="e")
                            nc.scalar.activation(e[:], s[:], AF.Exp, scale=0.125)
                            for hh, o in ((0, oA), (1, oB)):
                                nc.tensor.matmul(
                                    o[:],
                                    vaug[:, nkt, pr * 2 + hh, :],
                                    e[:, hh, :],
                                    start=(nkt == 0), stop=(nkt == NT - 1),
                                )
                        # transpose back to token-major + normalize + store
                        for hh, o in ((0, oA), (1, oB)):
                            h = pr * 2 + hh
                            osb = asb.tile([128, 512], f32r, tag="osb")
                            nc.vector.tensor_copy(osb[0:D + 1, :], o[:])
                            for sub in range(4):
                                ot = fps.tile([128, 128], fp32, tag="ot")
                                nc.tensor.transpose(
                                    ot[:].bitcast(f32r),
                                    osb[:, sub * 128:(sub + 1) * 128],
                                    idn[:],
                                )
                                rc = asb.tile([128, 1], fp32, tag="rc")
                                nc.vector.reciprocal(rc[:], ot[:, D:D + 1])
                                ob = asb.tile([128, D], fp32, tag="ob")
                                nc.vector.tensor_scalar_mul(ob[:], ot[:, 0:D], rc[:])
                                nt_o = nq * 4 + sub
                                nc.sync.dma_start(
                                    out_d[nt_o * 128:(nt_o + 1) * 128,
                                          h * D:(h + 1) * D],
                                    ob[:],
                                )
    nc.compile()
    return nc


def _prep_inputs(x, W_qkv, b_qkv, rot):
    """Build the 8 per-core input maps."""
    x = np.ascontiguousarray(x, np.float32)
    W = np.asarray(W_qkv, np.float32)
    bq = np.asarray(b_qkv, np.float32)
    rot = np.asarray(rot, np.float32).reshape(N, D // 2, 2, 2)

    # feature permutation for q/k: even pair-components first, then odd
    dperm = np.concatenate([np.arange(0, D, 2), np.arange(1, D, 2)])

    # rope tables, token-major, replicated across the 4 local heads
    A = rot[:, :, 0, 0]; Bc = rot[:, :, 0, 1]
    C = rot[:, :, 1, 0]; Dd = rot[:, :, 1, 1]
    ad1 = np.concatenate([A, Dd], axis=1)    # [N, 64]
    bc1 = np.concatenate([Bc, C], axis=1)
    adtab = np.tile(ad1, (1, HL)).astype(np.float32)   # [N, 256]
    bctab = np.tile(bc1, (1, HL)).astype(np.float32)
    idn = np.eye(128, dtype=np.float32)

    in_maps = []
    for core in range(8):
        b = core // 4
        g = core % 4
        heads = np.arange(g * HL, (g + 1) * HL)
        cols_q = np.concatenate(
            [h * (D * 3) + dperm * 3 + 0 for h in heads])
        cols_k = np.concatenate(
            [h * (D * 3) + dperm * 3 + 1 for h in heads])
        cols_v = np.concatenate(
            [h * (D * 3) + np.arange(D) * 3 + 2 for h in heads])
        cols = np.concatenate([cols_q, cols_k, cols_v])
        in_maps.append({
            "xT": np.ascontiguousarray(x[b].T),
            "w768": np.ascontiguousarray(W[:, cols]),
            "bias768": np.ascontiguousarray(bq[cols][None, :]),
            "adtab": adtab,
            "bctab": bctab,
            "idn": idn,
            "ones1": np.ones((1, 128), np.float32),
            "onescol": np.ones((128, NT * HL), np.float32),
        })
    return in_maps


def _run(in_maps, trace=False):
    global _COMPILED
    from concourse import bass_utils
    if _COMPILED is None:
        _COMPILED = _build_bass()
    return bass_utils.run_bass_kernel_spmd(
        _COMPILED, in_maps, list(range(8)), trace=trace)


def kernel(x, W_qkv, b_qkv, rot):
    in_maps = _prep_inputs(x, W_qkv, b_qkv, rot)
    res = _run(in_maps)
    out = np.empty((B, N, DIM), np.float32)
    for core in range(8):
        b = core // 4
        g = core % 4
        out[b, :, g * HL * D:(g + 1) * HL * D] = res.results[core]["out_local"]
    return out
